# revision 13
# baseline (speedup 1.0000x reference)
"""GAT (2-layer graph attention network + mean-pool + classifier) on 8 Trainium2
NeuronCores via Bass/Tile.

v4 design — the wall-clock is dominated by the per-call host->device upload
and D2D collective bytes, so both are minimized:
- Inputs are TWO tensors per core: xT (own-shard node features, slot-ordered,
  bf16, only used columns) and one packed f32 "aux" blob carrying weights,
  pooling metadata and the gather index table (int32 bit-cast into f32).
- Unified slot-grid row space: node (core c, partition p, column j) lives at
  table row c*16384 + p*128 + j for BOTH layer tables; L1/L2 share one batch
  geometry so one idx array serves both layers.
- Each core projects only its own shard for layer 1; shards are AllGathered
  (72 cols: h|al_s) into the bf16 gather table. al_d for own nodes comes from
  a small local side table via a direct strided DMA (no gather).
- For layer 2 the (smaller) x1 activations are AllGathered and every core
  projects the full table2 locally.
- Edge aggregation: degree-class slot layout, k-OUTER ordering -> segment
  reductions are dense pairwise plane adds (bf16+bf16->f32), no masks (pad
  slots point at a row whose attention logit is -300).
"""

import os
import sys
import numpy as np

sys.path.insert(0, "/opt/trn_rl_repo")

P = 128          # partitions
NG = 256         # graphs
NCLS = 10        # classes
NCORES = 8
NCOL = 128       # node columns per core
SHROWS = P * NCOL  # table rows per core shard (16384)

CLASSES = [1, 2, 3, 4, 5, 6, 7, 8, 10, 12, 14, 16, 18, 20, 22, 24, 26, 28,
           30, 32, 36, 40, 44, 48, 56, 64, 80, 96]

SB, NB = 128, 32   # batch: max slots, max node-columns (both layers)
PADROW = 127       # slot (p=0, j=127) of core 0; column 127 is kept invalid


def _aux_layout(meta):
    """Column offsets of the packed per-core aux tensor (f32 view)."""
    gmax, S = meta["gmax"], meta["S"]
    off = {}
    cur = 0
    for name, width in [("gid", NCOL), ("wval", NCOL), ("iota", gmax),
                        ("own", 1), ("b2g", 128), ("fcb", NCLS),
                        ("W1", 64), ("a1s", 64), ("a1d", 64), ("b1", 64),
                        ("W2", 128), ("a2s", 128), ("a2d", 128),
                        ("fcw", NCLS), ("idx", S)]:
        off[name] = cur
        cur += width
    off["_total"] = cur
    return off


# ----------------------------------------------------------------------------
# host-side preprocessing (numpy only; index/layout work, no model math)
# ----------------------------------------------------------------------------

def _mk_batches(active, G_w, col0):
    out = []
    ecur = 0
    class_e0 = {}
    class_step = {}
    for i in active:
        w = CLASSES[i]
        step = max(1, min(NB, SB // w))
        class_e0[i] = ecur
        class_step[i] = step
        j = 0
        while j < int(G_w[i]):
            nc_ = min(step, int(G_w[i]) - j)
            out.append((w, col0[i] + j, nc_, ecur))
            ecur += w * nc_
            j += nc_
    return out, ecur, class_e0, class_step


def _prep(x, edge_index, batch):
    N = x.shape[0]

    src = np.concatenate([edge_index[0], np.arange(N, dtype=np.int64)])
    dst = np.concatenate([edge_index[1], np.arange(N, dtype=np.int64)])
    batch = np.asarray(batch)

    gstart = np.searchsorted(batch, np.arange(NG), side="left")
    gend = np.searchsorted(batch, np.arange(NG), side="right")
    cum = gend.astype(np.float64)
    bounds = [0]
    for c in range(1, NCORES):
        bounds.append(int(np.searchsorted(cum, c * N / NCORES)))
    bounds.append(NG)
    g0 = np.array(bounds[:-1])
    g1 = np.array(bounds[1:])
    n0 = np.where(g0 < NG, gstart[np.minimum(g0, NG - 1)], N)
    n1 = np.where(g1 > 0, gend[np.minimum(g1 - 1, NG - 1)], 0)
    n0[0] = 0
    n1[-1] = N
    gmax = int((g1 - g0).max())

    order = np.argsort(dst, kind="stable")
    src_s, dst_s = src[order], dst[order]
    core_edges = []
    for c in range(NCORES):
        lo = np.searchsorted(dst_s, n0[c])
        hi = np.searchsorted(dst_s, n1[c])
        core_edges.append((src_s[lo:hi], dst_s[lo:hi] - n0[c]))

    cls_arr = np.array(CLASSES)
    counts = np.zeros((NCORES, len(CLASSES)), np.int64)
    degs = []
    for c in range(NCORES):
        nloc = int(n1[c] - n0[c])
        d = np.bincount(core_edges[c][1], minlength=nloc)
        assert d.min() >= 1 and d.max() <= CLASSES[-1], (d.min(), d.max())
        degs.append(d)
        ci = np.searchsorted(cls_arr, d)
        counts[c] = np.bincount(ci, minlength=len(CLASSES))
    G_w = np.maximum.reduce([(counts[c] + P - 1) // P for c in range(NCORES)])
    active = [i for i in range(len(CLASSES)) if counts[:, i].max() > 0]
    col0 = {}
    ncol_total = 0
    for i in active:
        col0[i] = ncol_total
        ncol_total += int(G_w[i])
    assert ncol_total <= NCOL - 1, ncol_total   # column 127 stays invalid

    batches, S, ce0, step = _mk_batches(active, G_w, col0)

    g_core = np.zeros(N, np.int32)
    g_p = np.zeros(N, np.int32)
    g_j = np.zeros(N, np.int32)
    per_core = []
    for c in range(NCORES):
        d = degs[c]
        ci = np.searchsorted(cls_arr, d)
        esrc, edst = core_edges[c]
        eorder = np.lexsort((esrc, edst))
        esrc = esrc[eorder]
        edst = edst[eorder]

        slot_node = np.full((P, NCOL), -1, np.int64)
        e_p = np.zeros(len(esrc), np.int64)
        e_col = np.zeros(len(esrc), np.int64)
        for i in active:
            w = CLASSES[i]
            nodes = np.nonzero(ci == i)[0]
            if len(nodes) == 0:
                continue
            s = np.arange(len(nodes))
            pp = s % P
            jrel = s // P
            slot_node[pp, col0[i] + jrel] = nodes
            g_core[n0[c] + nodes] = c
            g_p[n0[c] + nodes] = pp
            g_j[n0[c] + nodes] = col0[i] + jrel
            emask = ci[edst] == i
            eidx = np.nonzero(emask)[0]
            dn = d[nodes]
            t = np.repeat(s, dn)
            starts = np.concatenate([[0], np.cumsum(dn)[:-1]])
            k = np.arange(len(eidx)) - np.repeat(starts, dn)
            jr = jrel[t]
            q = jr // step[i]
            ncols_q = np.minimum(step[i], int(G_w[i]) - q * step[i])
            e_p[eidx] = pp[t]
            e_col[eidx] = (ce0[i] + w * q * step[i] + k * ncols_q
                           + (jr - q * step[i]))
        per_core.append(dict(slot_node=slot_node, esrc=esrc, e_p=e_p,
                             e_col=e_col))

    meta = dict(
        N=N, S=S, gmax=gmax, ncu=ncol_total, batches=batches,
        n0=n0.tolist(), n1=n1.tolist(), g0=g0.tolist(), g1=g1.tolist(),
    )

    host = []
    cnt = (gend - gstart).astype(np.float32)
    lay = _aux_layout(meta)
    for c in range(NCORES):
        pc = per_core[c]
        sl = pc["esrc"]
        row = (g_core[sl].astype(np.int64) * SHROWS
               + g_p[sl].astype(np.int64) * NCOL + g_j[sl])
        idx = np.full((P, S), PADROW, np.int32)
        idx[pc["e_p"], pc["e_col"]] = row.astype(np.int32)

        sn = pc["slot_node"]
        valid = sn >= 0
        nidx = np.where(valid, sn, 0)

        gnode = batch[np.minimum(nidx + n0[c], N - 1)]
        gl = (gnode - g0[c]).astype(np.int64)
        ok = valid & (gl >= 0) & (gl < gmax)
        gid = np.where(ok, gl, -1).astype(np.float32)
        wval = np.where(ok, 1.0 / np.maximum(cnt[np.minimum(gnode, NG - 1)],
                                             1.0), 0.0).astype(np.float32)
        ownbase = (np.arange(P, dtype=np.int32) * NCOL
                   + c * SHROWS).reshape(P, 1)
        host.append(dict(idx=idx, gid=gid, wval=wval, ownbase=ownbase,
                         valid=valid, nidx=nidx))

    aux_meta = dict(lay=lay)
    aux = dict(slot_nodes=[pc["slot_node"] for pc in per_core],
               g_core=g_core, g_p=g_p, g_j=g_j, host=host, aux_meta=aux_meta)
    return host, meta, aux


# ----------------------------------------------------------------------------
# program builder
# ----------------------------------------------------------------------------

def build_program(tc, ins, meta):
    import concourse.bass as bass
    import concourse.mybir as mybir
    from concourse.masks import make_identity

    nc = tc.nc
    dt = mybir.dt
    AX = mybir.AxisListType
    OP = mybir.AluOpType
    ACTF = mybir.ActivationFunctionType

    gmax = meta["gmax"]
    ncu = meta["ncu"]
    S = meta["S"]
    lay = _aux_layout(meta)
    R1, R1F, R2 = 72, 80, 130   # gather row widths; R1F = h|al_s|al_d

    t1shard = nc.dram_tensor("t1shard", [SHROWS, R1], dt.bfloat16, kind="Internal").ap()
    aldsh = nc.dram_tensor("aldsh", [SHROWS, 8], dt.bfloat16, kind="Internal").ap()
    table1 = nc.dram_tensor("table1", [SHROWS * NCORES, R1], dt.bfloat16, kind="Internal").ap()
    x1sh = nc.dram_tensor("x1sh", [64, SHROWS], dt.bfloat16, kind="Internal").ap()
    x1full = nc.dram_tensor("x1full", [64 * NCORES, SHROWS], dt.bfloat16, kind="Internal").ap()
    table2 = nc.dram_tensor("table2", [SHROWS * NCORES, R2], dt.bfloat16, kind="Internal").ap()

    if os.environ.get("GAT_NULL"):
        with tc.tile_pool(name="nullp", bufs=1) as np_:
            z = np_.tile([gmax, NCLS], dt.float32)
            nc.scalar.memzero(z[:])
            nc.sync.dma_start(out=ins["out"][:], in_=z[:])
        return

    with tc.tile_pool(name="cst", bufs=1) as cst:
        aux = cst.tile([P, lay["_total"]], dt.float32)
        nc.sync.dma_start(out=aux[:], in_=ins["aux"][:])

        def af(name, width, p0=0, pn=P):
            return aux[p0:pn, lay[name]:lay[name] + width]

        # ---------------- fused weights ----------------
        rhs1f = cst.tile([P, 80], dt.float32)
        nc.vector.tensor_copy(out=rhs1f[:, 0:64], in_=af("W1", 64))
        tmp1 = cst.tile([P, 64], dt.float32)
        nc.vector.tensor_tensor(out=tmp1[:], in0=af("W1", 64),
                                in1=af("a1s", 64), op=OP.mult)
        nc.vector.tensor_reduce(
            out=rhs1f[:, 64:72], in_=tmp1[:].rearrange("p (h c) -> p h c", c=8),
            axis=AX.X, op=OP.add)
        nc.vector.tensor_tensor(out=tmp1[:], in0=af("W1", 64),
                                in1=af("a1d", 64), op=OP.mult)
        nc.vector.tensor_reduce(
            out=rhs1f[:, 72:80], in_=tmp1[:].rearrange("p (h c) -> p h c", c=8),
            axis=AX.X, op=OP.add)
        rhs1 = cst.tile([P, 80], dt.bfloat16)
        nc.vector.tensor_copy(out=rhs1[:], in_=rhs1f[:])

        rhs2f = cst.tile([64, R2], dt.float32)
        nc.vector.tensor_copy(out=rhs2f[:, 0:128], in_=af("W2", 128, 0, 64))
        tmp2 = cst.tile([64, 128], dt.float32)
        nc.vector.tensor_tensor(out=tmp2[:], in0=af("W2", 128, 0, 64),
                                in1=af("a2s", 128, 0, 64), op=OP.mult)
        nc.vector.tensor_reduce(out=rhs2f[:, 128:129], in_=tmp2[:], axis=AX.X, op=OP.add)
        nc.vector.tensor_tensor(out=tmp2[:], in0=af("W2", 128, 0, 64),
                                in1=af("a2d", 128, 0, 64), op=OP.mult)
        nc.vector.tensor_reduce(out=rhs2f[:, 129:130], in_=tmp2[:], axis=AX.X, op=OP.add)
        rhs2 = cst.tile([64, R2], dt.bfloat16)
        nc.vector.tensor_copy(out=rhs2[:], in_=rhs2f[:])

        ident = cst.tile([P, P], dt.float32)
        make_identity(nc, ident[:])
        padc = cst.tile([1, 8], dt.bfloat16)
        nc.scalar.memzero(padc[:])
        nc.vector.tensor_scalar(out=padc[:], in0=padc[:], scalar1=-300.0,
                                scalar2=None, op0=OP.add)
        own = af("own", 1).bitcast(dt.int32)
        idxs = af("idx", S).bitcast(dt.int32)

        ald1 = cst.tile([P, NCOL * 8], dt.float32)
        ald2 = cst.tile([P, NCOL], dt.float32)

        with tc.tile_pool(name="slotp", bufs=1) as slotp:
            x1slot = slotp.tile([P, NCOL * 64], dt.float32)
            nc.scalar.memzero(x1slot[:])

            # ---------------- P1: own-shard L1 projection ----------------
            with tc.tile_pool(name="p1", bufs=3) as p1, \
                 tc.tile_pool(name="p1ps", bufs=4, space="PSUM") as p1ps:
                GT = 6
                t = 0
                while t < ncu:
                    g = min(GT, ncu - t)
                    xt = p1.tile([P, GT * P], dt.bfloat16, tag="xt")
                    nc.sync.dma_start(out=xt[:, :g * P],
                                      in_=ins["xT"][:, t * P:(t + g) * P])
                    ps = p1ps.tile([P, GT * R1F], dt.float32, tag="ps")
                    for i in range(g):
                        nc.tensor.matmul(out=ps[:, i * R1F:(i + 1) * R1F],
                                         lhsT=xt[:, i * P:(i + 1) * P],
                                         rhs=rhs1[:], start=True, stop=True)
                    st = p1.tile([P, GT * R1F], dt.bfloat16, tag="st")
                    nc.vector.tensor_copy(out=st[:, :g * R1F], in_=ps[:, :g * R1F])
                    stv = st[:, :g * R1F].rearrange("p (t r) -> p t r", r=R1F)
                    nc.sync.dma_start(
                        out=t1shard[:].rearrange("(p t) r -> p t r", p=P)[:, t:t + g],
                        in_=stv[:, :, 0:R1])
                    nc.sync.dma_start(
                        out=aldsh[:].rearrange("(p t) r -> p t r", p=P)[:, t:t + g],
                        in_=stv[:, :, R1:R1F])
                    t += g
                # zero the unused tail columns [ncu, 128)
                if ncu < NCOL:
                    zt = p1.tile([P, (NCOL - ncu) * R1], dt.bfloat16, tag="zt")
                    nc.scalar.memzero(zt[:])
                    nc.sync.dma_start(
                        out=t1shard[:].rearrange("(p t) r -> p (t r)", p=P)[:, ncu * R1:],
                        in_=zt[:])
            # pad row (local row 127 = slot (p=0, j=127), invalid by assert)
            nc.sync.dma_start(out=t1shard[PADROW:PADROW + 1, 64:72], in_=padc[0:1, :])

            # ---------------- AllGather table1 ----------------
            nc.gpsimd.collective_compute(
                "AllGather", mybir.AluOpType.bypass,
                replica_groups=[list(range(NCORES))],
                ins=[t1shard[:].opt()], outs=[table1[:].opt()])

            # al_d1 for own nodes: direct strided DMA from the local side table
            ald1b = cst.tile([P, NCOL * 8], dt.bfloat16)
            nc.sync.dma_start(
                out=ald1b[:, :ncu * 8],
                in_=aldsh[:].rearrange("(p t) r -> p (t r)", p=P)[:, :ncu * 8])
            nc.scalar.memzero(ald1[:])
            nc.vector.tensor_copy(out=ald1[:, :ncu * 8], in_=ald1b[:, :ncu * 8])

            # ---------------- P3: L1 edge phase ----------------
            if not os.environ.get("GAT_NOEDGE"):
                _edge_phase(tc, idxs, meta, layer=1, table=table1, ald=ald1,
                            out_slot=x1slot, wpool=None, pool_psum=None)

            # bias + relu
            nc.vector.tensor_tensor(
                out=x1slot[:].rearrange("p (n f) -> p n f", f=64),
                in0=x1slot[:].rearrange("p (n f) -> p n f", f=64),
                in1=af("b1", 64).rearrange("p (o f) -> p o f", o=1).to_broadcast([P, NCOL, 64]),
                op=OP.add)
            nc.scalar.activation(out=x1slot[:], in_=x1slot[:], func=ACTF.Relu)

            if os.environ.get("GAT_STOP"):
                nc.sync.dma_start(out=ins["out"][:, 0:1],
                                  in_=x1slot[0:gmax, 0:1])
                return

            # ------------- P4: transpose x1, AllGather x1 -------------
            with tc.tile_pool(name="p4", bufs=1) as p4:
                x1T = p4.tile([64, SHROWS], dt.bfloat16)
                with tc.tile_pool(name="p4ps", bufs=4, space="PSUM") as p4ps:
                    for j2 in range(0, NCOL, 2):
                        ps = p4ps.tile([64, 2 * P], dt.float32, tag="tp")
                        for k in range(2):
                            j = j2 + k
                            nc.tensor.transpose(
                                out=ps[:, k * P:(k + 1) * P],
                                in_=x1slot[:, j * 64:(j + 1) * 64], identity=ident[:])
                        nc.vector.tensor_copy(out=x1T[:, j2 * P:(j2 + 2) * P], in_=ps[:])
                nc.sync.dma_start(out=x1sh[:], in_=x1T[:])
        nc.gpsimd.collective_compute(
            "AllGather", mybir.AluOpType.bypass,
            replica_groups=[list(range(NCORES))],
            ins=[x1sh[:].opt()], outs=[x1full[:].opt()])

        # ---------------- P5: full local L2 projection ----------------
        t2v = table2[:].rearrange("(o p t) r -> p o (t r)", o=NCORES, p=P)
        with tc.tile_pool(name="p5", bufs=3) as p5, \
             tc.tile_pool(name="p5ps", bufs=4, space="PSUM") as p5ps:
            GL = 8
            GP = 3
            for o in range(NCORES):
                for jl in range(0, NCOL, GL):
                    blk = p5.tile([64, GL * P], dt.bfloat16, tag="blk")
                    nc.sync.dma_start(out=blk[:],
                                      in_=x1full[o * 64:(o + 1) * 64, jl * P:(jl + GL) * P])
                    jp = 0
                    while jp < GL:
                        gp = min(GP, GL - jp)
                        ps = p5ps.tile([P, GP * R2], dt.float32, tag="ps2")
                        for i in range(gp):
                            nc.tensor.matmul(
                                out=ps[:, i * R2:(i + 1) * R2],
                                lhsT=blk[:, (jp + i) * P:(jp + i + 1) * P],
                                rhs=rhs2[:], start=True, stop=True)
                        st = p5.tile([P, GP * R2], dt.bfloat16, tag="st2")
                        nc.vector.tensor_copy(out=st[:, :gp * R2], in_=ps[:, :gp * R2])
                        tt = jl + jp
                        nc.sync.dma_start(
                            out=(t2v[:, o:o + 1, tt * R2:(tt + gp) * R2]
                                 .rearrange("p o x -> p (o x)")),
                            in_=st[:, :gp * R2])
                        jp += gp
        nc.sync.dma_start(out=table2[PADROW:PADROW + 1, 128:129],
                          in_=padc[0:1, 0:1])

        # ---------------- P6: al_d2 for own nodes ----------------
        with tc.tile_pool(name="p6", bufs=1) as p6:
            tmp = p6.tile([P, NCOL * R2], dt.bfloat16)
            nc.gpsimd.indirect_dma_start(
                out=tmp[:], out_offset=None, in_=table2[:],
                in_offset=bass.IndirectOffsetOnAxis(ap=own[:, 0:1], axis=0))
            nc.vector.tensor_copy(
                out=ald2[:],
                in_=tmp[:].rearrange("p (n r) -> p n r", r=R2)[:, :, 129:130])

        if os.environ.get("GAT_STOP2"):
            nc.sync.dma_start(out=ins["out"][:, 0:1], in_=ald2[0:gmax, 0:1])
            return

        # ---------------- P7: L2 edge phase + pooling ----------------
        with tc.tile_pool(name="pool", bufs=1) as poolp, \
             tc.tile_pool(name="poolps", bufs=1, space="PSUM") as poolps:
            wpool = poolp.tile([P, NCOL * gmax], dt.float32)
            wpv = wpool[:].rearrange("p (n g) -> p n g", g=gmax)
            nc.vector.tensor_tensor(
                out=wpv,
                in0=(af("gid", NCOL).rearrange("p (n o) -> p n o", o=1)
                     .to_broadcast([P, NCOL, gmax])),
                in1=(af("iota", gmax).rearrange("p (o g) -> p o g", o=1)
                     .to_broadcast([P, NCOL, gmax])),
                op=OP.is_equal)
            nc.vector.tensor_tensor(
                out=wpv, in0=wpv,
                in1=(af("wval", NCOL).rearrange("p (n o) -> p n o", o=1)
                     .to_broadcast([P, NCOL, gmax])),
                op=OP.mult)

            pool_ps = poolps.tile([gmax, 128], dt.float32)
            _edge_phase(tc, idxs, meta, layer=2, table=table2, ald=ald2,
                        out_slot=None, wpool=wpool, pool_psum=pool_ps)

            # ---------------- P8: head ----------------
            pooled = poolp.tile([gmax, 128], dt.float32)
            nc.vector.tensor_copy(out=pooled[:], in_=pool_ps[:])
            nc.vector.tensor_tensor(out=pooled[:], in0=pooled[:],
                                    in1=af("b2g", 128, 0, gmax), op=OP.add)
            with tc.tile_pool(name="hps", bufs=1, space="PSUM") as hps:
                pT_ps = hps.tile([P, gmax], dt.float32)
                nc.tensor.transpose(out=pT_ps[:], in_=pooled[:],
                                    identity=ident[:gmax, :gmax])
                pT = poolp.tile([P, gmax], dt.float32)
                nc.vector.tensor_copy(out=pT[:], in_=pT_ps[:])
                lg_ps = hps.tile([gmax, NCLS], dt.float32)
                nc.tensor.matmul(out=lg_ps[:], lhsT=pT[:], rhs=af("fcw", NCLS),
                                 start=True, stop=True)
                lg = poolp.tile([gmax, NCLS], dt.float32)
                nc.vector.tensor_copy(out=lg[:], in_=lg_ps[:])
            nc.vector.tensor_tensor(out=lg[:], in0=lg[:],
                                    in1=af("fcb", NCLS, 0, gmax), op=OP.add)
            # log_softmax
            m = poolp.tile([gmax, 1], dt.float32)
            nc.vector.tensor_reduce(out=m[:], in_=lg[:], axis=AX.X, op=OP.max)
            nc.vector.tensor_scalar(out=lg[:], in0=lg[:], scalar1=m[:],
                                    scalar2=None, op0=OP.subtract)
            ex = poolp.tile([gmax, NCLS], dt.float32)
            nc.scalar.activation(out=ex[:], in_=lg[:], func=ACTF.Exp)
            ss = poolp.tile([gmax, 1], dt.float32)
            nc.vector.tensor_reduce(out=ss[:], in_=ex[:], axis=AX.X, op=OP.add)
            nc.scalar.activation(out=ss[:], in_=ss[:], func=ACTF.Ln)
            nc.vector.tensor_scalar(out=lg[:], in0=lg[:], scalar1=ss[:],
                                    scalar2=None, op0=OP.subtract)
            nc.sync.dma_start(out=ins["out"][:], in_=lg[:])


def _edge_phase(tc, idxs, meta, layer, table, ald, out_slot, wpool, pool_psum):
    import concourse.bass as bass
    import concourse.mybir as mybir

    nc = tc.nc
    dt = mybir.dt
    OP = mybir.AluOpType
    ACTF = mybir.ActivationFunctionType
    gmax = meta["gmax"]
    batches = meta["batches"]

    if layer == 1:
        R, F, H = 72, 64, 8
        HOFF = 64
    else:
        R, F, H = 130, 128, 1
        HOFF = 128
    C = F // H

    last = batches[-1]
    first = batches[0]

    with tc.tile_pool(name=f"ed{layer}", bufs=3 if layer == 1 else 2) as ep, \
         tc.tile_pool(name=f"eds{layer}", bufs=3) as eps:
        for (w, j0, ncols, ec0) in batches:
            Sb = ncols * w
            ed = ep.tile([P, SB * R], dt.bfloat16, tag="ed")
            for s in range(Sb):
                nc.gpsimd.indirect_dma_start(
                    out=ed[:, s * R:(s + 1) * R], out_offset=None, in_=table[:],
                    in_offset=bass.IndirectOffsetOnAxis(
                        ap=idxs[:, ec0 + s:ec0 + s + 1], axis=0))

            edk = ed[:, :Sb * R].rearrange("p (k n r) -> p k n r", k=w, r=R)
            eds_v = ed[:, :Sb * R].rearrange("p (s r) -> p s r", r=R)
            # e = al_s[src] + al_d[dst] (al_d identical across the w planes)
            et = eps.tile([P, SB * H], dt.float32, tag="et")
            etv = et[:, :Sb * H]
            nc.vector.tensor_copy(
                out=etv.rearrange("p (s h) -> p s h", h=H),
                in_=eds_v[:, :, HOFF:HOFF + H])
            aldv = (ald[:].rearrange("p (n h) -> p n h", h=H)[:, j0:j0 + ncols]
                    .rearrange("p (o n) h -> p o n h", o=1)
                    .to_broadcast([P, w, ncols, H]))
            nc.vector.tensor_tensor(
                out=etv.rearrange("p (k n h) -> p k n h", k=w, h=H),
                in0=etv.rearrange("p (k n h) -> p k n h", k=w, h=H),
                in1=aldv, op=OP.add)
            # exp(leaky_relu(e))  (leaky = max(x, 0.2x))
            lt = eps.tile([P, SB * H], dt.float32, tag="lt")
            nc.vector.tensor_scalar(out=lt[:, :Sb * H], in0=etv, scalar1=0.2,
                                    scalar2=None, op0=OP.mult)
            nc.vector.tensor_tensor(out=etv, in0=etv, in1=lt[:, :Sb * H], op=OP.max)
            nc.scalar.activation(out=etv, in_=etv, func=ACTF.Exp)
            # s[d] = sum_k exp : dense plane adds (f32)
            NH = ncols * H
            s = eps.tile([P, NB * 8], dt.float32, tag="s")
            sv = s[:, :NH]
            if w == 1:
                nc.vector.tensor_scalar(out=sv, in0=et[:, :NH], scalar1=1e-16,
                                        scalar2=None, op0=OP.add)
            else:
                nc.vector.tensor_tensor(out=sv, in0=et[:, 0:NH],
                                        in1=et[:, NH:2 * NH], op=OP.add)
                for k in range(2, w):
                    nc.vector.tensor_tensor(out=sv, in0=sv,
                                            in1=et[:, k * NH:(k + 1) * NH],
                                            op=OP.add)
                nc.vector.tensor_scalar(out=sv, in0=sv, scalar1=1e-16,
                                        scalar2=None, op0=OP.add)
            nc.vector.reciprocal(out=sv, in_=sv)
            # alpha in bf16 for the h multiply
            etb = eps.tile([P, SB * H], dt.bfloat16, tag="etb")
            nc.vector.tensor_copy(out=etb[:, :Sb * H], in_=etv)
            # WH = h[src] * alpha (in place, bf16)
            if H == 1:
                hview = eds_v[:, :, 0:F]
                exv = (etb[:, :Sb].rearrange("p (s o) -> p s o", o=1)
                       .to_broadcast([P, Sb, F]))
            else:
                hview = eds_v[:, :, 0:F].rearrange("p s (h c) -> p s h c", h=H)
                exv = (etb[:, :Sb * H].rearrange("p (s h o) -> p s h o", h=H, o=1)
                       .to_broadcast([P, Sb, H, C]))
            nc.vector.tensor_tensor(out=hview, in0=hview, in1=exv, op=OP.mult)

            # out[d] = (sum_k WH) / s[d] : pairwise bf16+bf16->f32 plane adds
            if layer == 1:
                ov = (out_slot[:].rearrange("p (n f) -> p n f", f=F)
                      [:, j0:j0 + ncols])
                x2b = None
            else:
                x2b = ep.tile([P, NB * F], dt.float32, tag="x2b")
                ov = x2b[:, :ncols * F].rearrange("p (n f) -> p n f", f=F)

            def plane(k):
                return (edk[:, k:k + 1, :, 0:F]
                        .rearrange("p o n r -> p (o n) r"))
            acc = eps.tile([P, NB * F], dt.float32, tag="acc")
            av = acc[:, :ncols * F].rearrange("p (n f) -> p n f", f=F)
            if w == 1:
                nc.vector.tensor_copy(out=ov, in_=plane(0))
            else:
                nc.vector.tensor_tensor(out=ov, in0=plane(0), in1=plane(1),
                                        op=OP.add)
                k = 2
                while k + 1 < w:
                    nc.vector.tensor_tensor(out=av, in0=plane(k),
                                            in1=plane(k + 1), op=OP.add)
                    nc.vector.tensor_tensor(out=ov, in0=ov, in1=av, op=OP.add)
                    k += 2
                if k < w:
                    nc.vector.tensor_copy(out=av, in_=plane(k))
                    nc.vector.tensor_tensor(out=ov, in0=ov, in1=av, op=OP.add)
            if H == 1:
                sinvv = (sv.rearrange("p (n o) -> p n o", o=1)
                         .to_broadcast([P, ncols, F]))
                ovv = ov
            else:
                sinvv = (sv.rearrange("p (n h o) -> p n h o", h=H, o=1)
                         .to_broadcast([P, ncols, H, C]))
                ovv = ov.rearrange("p n (h c) -> p n h c", h=H)
            nc.vector.tensor_tensor(out=ovv, in0=ovv, in1=sinvv, op=OP.mult)

            if layer == 2:
                for jj in range(ncols):
                    nc.tensor.matmul(
                        out=pool_psum[:],
                        lhsT=wpool[:, (j0 + jj) * gmax:(j0 + jj + 1) * gmax],
                        rhs=x2b[:, jj * F:(jj + 1) * F],
                        start=((w, j0, ncols, ec0) == first and jj == 0),
                        stop=((w, j0, ncols, ec0) == last and jj == ncols - 1),
                        skip_group_check=True)


# ----------------------------------------------------------------------------
# runner
# ----------------------------------------------------------------------------

_CACHE = {}


def _get_nc(meta, in_map0):
    key = str(sorted(meta.items(), key=lambda kv: kv[0]))
    if key in _CACHE:
        return _CACHE[key]
    import concourse.bacc as bacc
    import concourse.tile as tile
    import concourse.mybir as mybir
    dt = mybir.dt
    nc = bacc.Bacc("TRN2", target_bir_lowering=False, debug=False,
                   num_devices=NCORES)
    ins = {}
    for name, arr in in_map0.items():
        ins[name] = nc.dram_tensor(name, list(arr.shape),
                                   _np_dtype_to_bir(arr),
                                   kind="ExternalInput").ap()
    ins["out"] = nc.dram_tensor("out", [meta["gmax"], NCLS], dt.float32,
                                kind="ExternalOutput").ap()
    with tile.TileContext(nc) as tc:
        build_program(tc, ins, meta)
    nc.compile()
    _CACHE[key] = nc
    return nc


def _np_dtype_to_bir(a):
    import concourse.mybir as mybir
    import ml_dtypes
    dt = mybir.dt
    if a.dtype == np.int32:
        return dt.int32
    if a.dtype == ml_dtypes.bfloat16:
        return dt.bfloat16
    return dt.float32


def make_inputs(x, edge_index, batch, W1, a_src1, a_dst1, b1, W2, a_src2,
                a_dst2, b2, fc_w, fc_b):
    import ml_dtypes
    x = np.asarray(x, np.float32)
    host, meta, auxd = _prep(x, np.asarray(edge_index), np.asarray(batch))
    gmax = meta["gmax"]
    ncu = meta["ncu"]
    lay = _aux_layout(meta)
    n0 = meta["n0"]

    ge = np.searchsorted(np.asarray(batch), np.arange(NG), side="left")
    gEnd = np.searchsorted(np.asarray(batch), np.arange(NG), side="right")

    in_maps = []
    for c in range(NCORES):
        h = host[c]
        aux = np.zeros((P, lay["_total"]), np.float32)

        def put(name, arr, p0=0):
            arr = np.asarray(arr, np.float32)
            aux[p0:p0 + arr.shape[0], lay[name]:lay[name] + arr.shape[1]] = arr

        put("gid", h["gid"])
        put("wval", h["wval"])
        put("iota", np.tile(np.arange(gmax, dtype=np.float32).reshape(1, gmax),
                            (P, 1)))
        aux[:, lay["own"]:lay["own"] + 1] = h["ownbase"].view(np.float32)
        g0c, g1c = meta["g0"][c], meta["g1"][c]
        nonempty = np.zeros((gmax, 1), np.float32)
        cntc = (gEnd - ge)[g0c:g1c]
        nonempty[:g1c - g0c, 0] = (cntc > 0).astype(np.float32)
        put("b2g", nonempty * np.asarray(b2, np.float32).reshape(1, 128))
        put("fcb", np.tile(np.asarray(fc_b, np.float32).reshape(1, NCLS),
                           (gmax, 1)))
        put("W1", np.asarray(W1, np.float32))
        put("a1s", np.tile(np.asarray(a_src1, np.float32).reshape(1, 64), (P, 1)))
        put("a1d", np.tile(np.asarray(a_dst1, np.float32).reshape(1, 64), (P, 1)))
        put("b1", np.tile(np.asarray(b1, np.float32).reshape(1, 64), (P, 1)))
        put("W2", np.asarray(W2, np.float32))
        put("a2s", np.tile(np.asarray(a_src2, np.float32).reshape(1, 128), (64, 1)))
        put("a2d", np.tile(np.asarray(a_dst2, np.float32).reshape(1, 128), (64, 1)))
        put("fcw", np.asarray(fc_w, np.float32))
        aux[:, lay["idx"]:lay["idx"] + meta["S"]] = h["idx"].view(np.float32)

        # x permuted into slot order, bf16, [feat, slot], used columns only
        sn = auxd["slot_nodes"][c]
        valid = sn >= 0
        xs = np.zeros((ncu * P, x.shape[1]), np.float32)
        ppi, jji = np.nonzero(valid)
        xs[jji * P + ppi] = x[n0[c] + sn[ppi, jji]]
        xT = np.ascontiguousarray(xs.T).astype(ml_dtypes.bfloat16)

        in_maps.append(dict(xT=xT, aux=aux))
    return in_maps, meta, auxd


def kernel(x, edge_index, batch, W1, a_src1, a_dst1, b1, W2, a_src2, a_dst2,
           b2, fc_w, fc_b):
    in_maps, meta, auxd = make_inputs(x, edge_index, batch, W1, a_src1, a_dst1,
                                      b1, W2, a_src2, a_dst2, b2, fc_w, fc_b)
    global _LAST
    _LAST = dict(meta=meta, aux=auxd)
    nc = _get_nc(meta, in_maps[0])
    from concourse.bass_utils import run_bass_kernel_spmd
    res = run_bass_kernel_spmd(nc, in_maps, core_ids=list(range(NCORES)))
    _LAST["res"] = res
    out = np.zeros((NG, NCLS), np.float32)
    for c in range(NCORES):
        g0, g1 = meta["g0"][c], meta["g1"][c]
        out[g0:g1] = res.results[c]["out"][:g1 - g0]
    return out


# revision 14
# speedup vs baseline: 2.0586x; 2.0586x over previous
"""GAT (2-layer graph attention network + mean-pool + classifier) on 8 Trainium2
NeuronCores via Bass/Tile.

v4 design — the wall-clock is dominated by the per-call host->device upload
and D2D collective bytes, so both are minimized:
- Inputs are TWO tensors per core: xT (own-shard node features, slot-ordered,
  bf16, only used columns) and one packed f32 "aux" blob carrying weights,
  pooling metadata and the gather index table (int32 bit-cast into f32).
- Unified slot-grid row space: node (core c, partition p, column j) lives at
  table row c*16384 + p*128 + j for BOTH layer tables; L1/L2 share one batch
  geometry so one idx array serves both layers.
- Each core projects only its own shard for layer 1; shards are AllGathered
  (72 cols: h|al_s) into the bf16 gather table. al_d for own nodes comes from
  a small local side table via a direct strided DMA (no gather).
- For layer 2 the (smaller) x1 activations are AllGathered and every core
  projects the full table2 locally.
- Edge aggregation: degree-class slot layout, k-OUTER ordering -> segment
  reductions are dense pairwise plane adds (bf16+bf16->f32), no masks (pad
  slots point at a row whose attention logit is -300).
"""

import os
import sys
import numpy as np

sys.path.insert(0, "/opt/trn_rl_repo")

P = 128          # partitions
NG = 256         # graphs
NCLS = 10        # classes
NCORES = 8
NCOL = 128       # node columns per core
SHROWS = P * NCOL  # table rows per core shard (16384)

CLASSES = [1, 2, 3, 4, 5, 6, 7, 8, 10, 12, 14, 16, 18, 20, 22, 24, 26, 28,
           30, 32, 36, 40, 44, 48, 56, 64, 80, 96]

SB, NB = 128, 32   # batch: max slots, max node-columns (both layers)
PADROW = 127       # slot (p=0, j=127) of core 0; column 127 is kept invalid


def _aux_layout(meta):
    """Column offsets of the packed per-core aux tensor (f32 view)."""
    gmax, S = meta["gmax"], meta["S"]
    off = {}
    cur = 0
    for name, width in [("gid", NCOL), ("wval", NCOL), ("iota", gmax),
                        ("own", 1), ("b2g", 128), ("fcb", NCLS),
                        ("W1", 64), ("a1s", 64), ("a1d", 64), ("b1", 64),
                        ("W2", 128), ("a2s", 128), ("a2d", 128),
                        ("fcw", NCLS), ("idx", S)]:
        off[name] = cur
        cur += width
    off["_total"] = cur
    return off


# ----------------------------------------------------------------------------
# host-side preprocessing (numpy only; index/layout work, no model math)
# ----------------------------------------------------------------------------

def _mk_batches(active, G_w, col0):
    out = []
    ecur = 0
    class_e0 = {}
    class_step = {}
    for i in active:
        w = CLASSES[i]
        step = max(1, min(NB, SB // w))
        class_e0[i] = ecur
        class_step[i] = step
        j = 0
        while j < int(G_w[i]):
            nc_ = min(step, int(G_w[i]) - j)
            out.append((w, col0[i] + j, nc_, ecur))
            ecur += w * nc_
            j += nc_
    return out, ecur, class_e0, class_step


def _prep(x, edge_index, batch):
    N = x.shape[0]

    src = np.concatenate([edge_index[0], np.arange(N, dtype=np.int64)])
    dst = np.concatenate([edge_index[1], np.arange(N, dtype=np.int64)])
    batch = np.asarray(batch)

    gstart = np.searchsorted(batch, np.arange(NG), side="left")
    gend = np.searchsorted(batch, np.arange(NG), side="right")
    cum = gend.astype(np.float64)
    bounds = [0]
    for c in range(1, NCORES):
        bounds.append(int(np.searchsorted(cum, c * N / NCORES)))
    bounds.append(NG)
    g0 = np.array(bounds[:-1])
    g1 = np.array(bounds[1:])
    n0 = np.where(g0 < NG, gstart[np.minimum(g0, NG - 1)], N)
    n1 = np.where(g1 > 0, gend[np.minimum(g1 - 1, NG - 1)], 0)
    n0[0] = 0
    n1[-1] = N
    gmax = int((g1 - g0).max())

    order = np.argsort(dst, kind="stable")
    src_s, dst_s = src[order], dst[order]
    core_edges = []
    for c in range(NCORES):
        lo = np.searchsorted(dst_s, n0[c])
        hi = np.searchsorted(dst_s, n1[c])
        core_edges.append((src_s[lo:hi], dst_s[lo:hi] - n0[c]))

    cls_arr = np.array(CLASSES)
    counts = np.zeros((NCORES, len(CLASSES)), np.int64)
    degs = []
    for c in range(NCORES):
        nloc = int(n1[c] - n0[c])
        d = np.bincount(core_edges[c][1], minlength=nloc)
        assert d.min() >= 1 and d.max() <= CLASSES[-1], (d.min(), d.max())
        degs.append(d)
        ci = np.searchsorted(cls_arr, d)
        counts[c] = np.bincount(ci, minlength=len(CLASSES))
    G_w = np.maximum.reduce([(counts[c] + P - 1) // P for c in range(NCORES)])
    active = [i for i in range(len(CLASSES)) if counts[:, i].max() > 0]
    col0 = {}
    ncol_total = 0
    for i in active:
        col0[i] = ncol_total
        ncol_total += int(G_w[i])
    assert ncol_total <= NCOL - 1, ncol_total   # column 127 stays invalid

    batches, S, ce0, step = _mk_batches(active, G_w, col0)

    g_core = np.zeros(N, np.int32)
    g_p = np.zeros(N, np.int32)
    g_j = np.zeros(N, np.int32)
    per_core = []
    for c in range(NCORES):
        d = degs[c]
        ci = np.searchsorted(cls_arr, d)
        esrc, edst = core_edges[c]
        eorder = np.lexsort((esrc, edst))
        esrc = esrc[eorder]
        edst = edst[eorder]

        slot_node = np.full((P, NCOL), -1, np.int64)
        e_p = np.zeros(len(esrc), np.int64)
        e_col = np.zeros(len(esrc), np.int64)
        for i in active:
            w = CLASSES[i]
            nodes = np.nonzero(ci == i)[0]
            if len(nodes) == 0:
                continue
            s = np.arange(len(nodes))
            pp = s % P
            jrel = s // P
            slot_node[pp, col0[i] + jrel] = nodes
            g_core[n0[c] + nodes] = c
            g_p[n0[c] + nodes] = pp
            g_j[n0[c] + nodes] = col0[i] + jrel
            emask = ci[edst] == i
            eidx = np.nonzero(emask)[0]
            dn = d[nodes]
            t = np.repeat(s, dn)
            starts = np.concatenate([[0], np.cumsum(dn)[:-1]])
            k = np.arange(len(eidx)) - np.repeat(starts, dn)
            jr = jrel[t]
            q = jr // step[i]
            ncols_q = np.minimum(step[i], int(G_w[i]) - q * step[i])
            e_p[eidx] = pp[t]
            e_col[eidx] = (ce0[i] + w * q * step[i] + k * ncols_q
                           + (jr - q * step[i]))
        per_core.append(dict(slot_node=slot_node, esrc=esrc, e_p=e_p,
                             e_col=e_col))

    meta = dict(
        N=N, S=S, gmax=gmax, ncu=ncol_total, batches=batches,
        n0=n0.tolist(), n1=n1.tolist(), g0=g0.tolist(), g1=g1.tolist(),
    )

    host = []
    cnt = (gend - gstart).astype(np.float32)
    lay = _aux_layout(meta)
    for c in range(NCORES):
        pc = per_core[c]
        sl = pc["esrc"]
        row = (g_core[sl].astype(np.int64) * SHROWS
               + g_p[sl].astype(np.int64) * NCOL + g_j[sl])
        idx = np.full((P, S), PADROW, np.int32)
        idx[pc["e_p"], pc["e_col"]] = row.astype(np.int32)

        sn = pc["slot_node"]
        valid = sn >= 0
        nidx = np.where(valid, sn, 0)

        gnode = batch[np.minimum(nidx + n0[c], N - 1)]
        gl = (gnode - g0[c]).astype(np.int64)
        ok = valid & (gl >= 0) & (gl < gmax)
        gid = np.where(ok, gl, -1).astype(np.float32)
        wval = np.where(ok, 1.0 / np.maximum(cnt[np.minimum(gnode, NG - 1)],
                                             1.0), 0.0).astype(np.float32)
        ownbase = (np.arange(P, dtype=np.int32) * NCOL
                   + c * SHROWS).reshape(P, 1)
        host.append(dict(idx=idx, gid=gid, wval=wval, ownbase=ownbase,
                         valid=valid, nidx=nidx))

    aux_meta = dict(lay=lay)
    aux = dict(slot_nodes=[pc["slot_node"] for pc in per_core],
               g_core=g_core, g_p=g_p, g_j=g_j, host=host, aux_meta=aux_meta)
    return host, meta, aux


# ----------------------------------------------------------------------------
# program builder
# ----------------------------------------------------------------------------

def build_program(tc, ins, meta):
    import concourse.bass as bass
    import concourse.mybir as mybir
    from concourse.masks import make_identity

    nc = tc.nc
    dt = mybir.dt
    AX = mybir.AxisListType
    OP = mybir.AluOpType
    ACTF = mybir.ActivationFunctionType

    gmax = meta["gmax"]
    ncu = meta["ncu"]
    S = meta["S"]
    lay = _aux_layout(meta)
    R1, R1F, R2 = 72, 80, 130   # gather row widths; R1F = h|al_s|al_d

    t1shard = nc.dram_tensor("t1shard", [SHROWS, R1], dt.bfloat16, kind="Internal").ap()
    aldsh = nc.dram_tensor("aldsh", [SHROWS, 8], dt.bfloat16, kind="Internal").ap()
    table1 = nc.dram_tensor("table1", [SHROWS * NCORES, R1], dt.bfloat16, kind="Internal").ap()
    x1sh = nc.dram_tensor("x1sh", [64, SHROWS], dt.bfloat16, kind="Internal").ap()
    x1fullA = nc.dram_tensor("x1fullA", [64 * NCORES, SHROWS // 2], dt.bfloat16, kind="Internal").ap()
    x1fullB = nc.dram_tensor("x1fullB", [64 * NCORES, SHROWS // 2], dt.bfloat16, kind="Internal").ap()
    table2 = nc.dram_tensor("table2", [SHROWS * NCORES, R2], dt.bfloat16, kind="Internal").ap()

    if os.environ.get("GAT_NULL"):
        with tc.tile_pool(name="nullp", bufs=1) as np_:
            z = np_.tile([gmax, NCLS], dt.float32)
            nc.scalar.memzero(z[:])
            nc.sync.dma_start(out=ins["out"][:], in_=z[:])
        return

    with tc.tile_pool(name="cst", bufs=1) as cst:
        aux = cst.tile([P, lay["_total"]], dt.float32)
        nc.sync.dma_start(out=aux[:], in_=ins["aux"][:])

        def af(name, width, p0=0, pn=P):
            return aux[p0:pn, lay[name]:lay[name] + width]

        # ---------------- fused weights ----------------
        rhs1f = cst.tile([P, 80], dt.float32)
        nc.vector.tensor_copy(out=rhs1f[:, 0:64], in_=af("W1", 64))
        tmp1 = cst.tile([P, 64], dt.float32)
        nc.vector.tensor_tensor(out=tmp1[:], in0=af("W1", 64),
                                in1=af("a1s", 64), op=OP.mult)
        nc.vector.tensor_reduce(
            out=rhs1f[:, 64:72], in_=tmp1[:].rearrange("p (h c) -> p h c", c=8),
            axis=AX.X, op=OP.add)
        nc.vector.tensor_tensor(out=tmp1[:], in0=af("W1", 64),
                                in1=af("a1d", 64), op=OP.mult)
        nc.vector.tensor_reduce(
            out=rhs1f[:, 72:80], in_=tmp1[:].rearrange("p (h c) -> p h c", c=8),
            axis=AX.X, op=OP.add)
        rhs1 = cst.tile([P, 80], dt.bfloat16)
        nc.vector.tensor_copy(out=rhs1[:], in_=rhs1f[:])

        rhs2f = cst.tile([64, R2], dt.float32)
        nc.vector.tensor_copy(out=rhs2f[:, 0:128], in_=af("W2", 128, 0, 64))
        tmp2 = cst.tile([64, 128], dt.float32)
        nc.vector.tensor_tensor(out=tmp2[:], in0=af("W2", 128, 0, 64),
                                in1=af("a2s", 128, 0, 64), op=OP.mult)
        nc.vector.tensor_reduce(out=rhs2f[:, 128:129], in_=tmp2[:], axis=AX.X, op=OP.add)
        nc.vector.tensor_tensor(out=tmp2[:], in0=af("W2", 128, 0, 64),
                                in1=af("a2d", 128, 0, 64), op=OP.mult)
        nc.vector.tensor_reduce(out=rhs2f[:, 129:130], in_=tmp2[:], axis=AX.X, op=OP.add)
        rhs2 = cst.tile([64, R2], dt.bfloat16)
        nc.vector.tensor_copy(out=rhs2[:], in_=rhs2f[:])

        ident = cst.tile([P, P], dt.float32)
        make_identity(nc, ident[:])
        padc = cst.tile([1, 8], dt.bfloat16)
        nc.scalar.memzero(padc[:])
        nc.vector.tensor_scalar(out=padc[:], in0=padc[:], scalar1=-300.0,
                                scalar2=None, op0=OP.add)
        own = af("own", 1).bitcast(dt.int32)
        idxs = af("idx", S).bitcast(dt.int32)

        ald1 = cst.tile([P, NCOL * 8], dt.float32)
        ald2 = cst.tile([P, NCOL], dt.float32)

        with tc.tile_pool(name="slotp", bufs=1) as slotp:
            x1slot = slotp.tile([P, NCOL * 64], dt.float32)
            nc.scalar.memzero(x1slot[:])

            # ---------------- P1: own-shard L1 projection ----------------
            with tc.tile_pool(name="p1", bufs=3) as p1, \
                 tc.tile_pool(name="p1ps", bufs=4, space="PSUM") as p1ps:
                GT = 6
                t = 0
                while t < ncu:
                    g = min(GT, ncu - t)
                    xt = p1.tile([P, GT * P], dt.bfloat16, tag="xt")
                    nc.sync.dma_start(out=xt[:, :g * P],
                                      in_=ins["xT"][:, t * P:(t + g) * P])
                    ps = p1ps.tile([P, GT * R1F], dt.float32, tag="ps")
                    for i in range(g):
                        nc.tensor.matmul(out=ps[:, i * R1F:(i + 1) * R1F],
                                         lhsT=xt[:, i * P:(i + 1) * P],
                                         rhs=rhs1[:], start=True, stop=True)
                    st = p1.tile([P, GT * R1F], dt.bfloat16, tag="st")
                    nc.vector.tensor_copy(out=st[:, :g * R1F], in_=ps[:, :g * R1F])
                    stv = st[:, :g * R1F].rearrange("p (t r) -> p t r", r=R1F)
                    nc.sync.dma_start(
                        out=t1shard[:].rearrange("(p t) r -> p t r", p=P)[:, t:t + g],
                        in_=stv[:, :, 0:R1])
                    nc.sync.dma_start(
                        out=aldsh[:].rearrange("(p t) r -> p t r", p=P)[:, t:t + g],
                        in_=stv[:, :, R1:R1F])
                    t += g
                # zero the unused tail columns [ncu, 128)
                if ncu < NCOL:
                    zt = p1.tile([P, (NCOL - ncu) * R1], dt.bfloat16, tag="zt")
                    nc.scalar.memzero(zt[:])
                    nc.sync.dma_start(
                        out=t1shard[:].rearrange("(p t) r -> p (t r)", p=P)[:, ncu * R1:],
                        in_=zt[:])
            # pad row (local row 127 = slot (p=0, j=127), invalid by assert)
            nc.sync.dma_start(out=t1shard[PADROW:PADROW + 1, 64:72], in_=padc[0:1, :])

            # ---------------- AllGather table1 ----------------
            nc.gpsimd.collective_compute(
                "AllGather", mybir.AluOpType.bypass,
                replica_groups=[list(range(NCORES))],
                ins=[t1shard[:].opt()], outs=[table1[:].opt()])

            # al_d1 for own nodes: direct strided DMA from the local side table
            ald1b = cst.tile([P, NCOL * 8], dt.bfloat16)
            nc.sync.dma_start(
                out=ald1b[:, :ncu * 8],
                in_=aldsh[:].rearrange("(p t) r -> p (t r)", p=P)[:, :ncu * 8])
            nc.scalar.memzero(ald1[:])
            nc.vector.tensor_copy(out=ald1[:, :ncu * 8], in_=ald1b[:, :ncu * 8])

            # ---------------- P3: L1 edge phase ----------------
            if not os.environ.get("GAT_NOEDGE"):
                _edge_phase(tc, idxs, meta, layer=1, table=table1, ald=ald1,
                            out_slot=x1slot, wpool=None, pool_psum=None)

            # bias + relu
            nc.vector.tensor_tensor(
                out=x1slot[:].rearrange("p (n f) -> p n f", f=64),
                in0=x1slot[:].rearrange("p (n f) -> p n f", f=64),
                in1=af("b1", 64).rearrange("p (o f) -> p o f", o=1).to_broadcast([P, NCOL, 64]),
                op=OP.add)
            nc.scalar.activation(out=x1slot[:], in_=x1slot[:], func=ACTF.Relu)

            if os.environ.get("GAT_STOP"):
                nc.sync.dma_start(out=ins["out"][:, 0:1],
                                  in_=x1slot[0:gmax, 0:1])
                return

            # ------------- P4: transpose x1, AllGather x1 -------------
            with tc.tile_pool(name="p4", bufs=1) as p4:
                x1T = p4.tile([64, SHROWS], dt.bfloat16)
                with tc.tile_pool(name="p4ps", bufs=4, space="PSUM") as p4ps:
                    for j2 in range(0, NCOL, 2):
                        ps = p4ps.tile([64, 2 * P], dt.float32, tag="tp")
                        for k in range(2):
                            j = j2 + k
                            nc.tensor.transpose(
                                out=ps[:, k * P:(k + 1) * P],
                                in_=x1slot[:, j * 64:(j + 1) * 64], identity=ident[:])
                        nc.vector.tensor_copy(out=x1T[:, j2 * P:(j2 + 2) * P], in_=ps[:])
                nc.sync.dma_start(out=x1sh[:], in_=x1T[:])
        nc.gpsimd.collective_compute(
            "AllGather", mybir.AluOpType.bypass,
            replica_groups=[list(range(NCORES))],
            ins=[x1sh[:, 0:SHROWS // 2].opt()], outs=[x1fullA[:].opt()])
        nc.gpsimd.collective_compute(
            "AllGather", mybir.AluOpType.bypass,
            replica_groups=[list(range(NCORES))],
            ins=[x1sh[:, SHROWS // 2:].opt()], outs=[x1fullB[:].opt()])

        # ---------------- P5: full local L2 projection ----------------
        t2v = table2[:].rearrange("(o p t) r -> p o (t r)", o=NCORES, p=P)
        with tc.tile_pool(name="p5", bufs=3) as p5, \
             tc.tile_pool(name="p5ps", bufs=4, space="PSUM") as p5ps:
            GL = 8
            GP = 3
            for half, xf in ((0, x1fullA), (1, x1fullB)):
                for o in range(NCORES):
                    for jl in range(0, NCOL // 2, GL):
                        blk = p5.tile([64, GL * P], dt.bfloat16, tag="blk")
                        nc.sync.dma_start(out=blk[:],
                                          in_=xf[o * 64:(o + 1) * 64, jl * P:(jl + GL) * P])
                        jp = 0
                        while jp < GL:
                            gp = min(GP, GL - jp)
                            ps = p5ps.tile([P, GP * R2], dt.float32, tag="ps2")
                            for i in range(gp):
                                nc.tensor.matmul(
                                    out=ps[:, i * R2:(i + 1) * R2],
                                    lhsT=blk[:, (jp + i) * P:(jp + i + 1) * P],
                                    rhs=rhs2[:], start=True, stop=True)
                            st = p5.tile([P, GP * R2], dt.bfloat16, tag="st2")
                            nc.vector.tensor_copy(out=st[:, :gp * R2], in_=ps[:, :gp * R2])
                            tt = half * (NCOL // 2) + jl + jp
                            nc.sync.dma_start(
                                out=(t2v[:, o:o + 1, tt * R2:(tt + gp) * R2]
                                     .rearrange("p o x -> p (o x)")),
                                in_=st[:, :gp * R2])
                            jp += gp
        nc.sync.dma_start(out=table2[PADROW:PADROW + 1, 128:129],
                          in_=padc[0:1, 0:1])

        # ---------------- P6: al_d2 for own nodes ----------------
        with tc.tile_pool(name="p6", bufs=1) as p6:
            tmp = p6.tile([P, NCOL * R2], dt.bfloat16)
            nc.gpsimd.indirect_dma_start(
                out=tmp[:], out_offset=None, in_=table2[:],
                in_offset=bass.IndirectOffsetOnAxis(ap=own[:, 0:1], axis=0))
            nc.vector.tensor_copy(
                out=ald2[:],
                in_=tmp[:].rearrange("p (n r) -> p n r", r=R2)[:, :, 129:130])

        if os.environ.get("GAT_STOP2"):
            nc.sync.dma_start(out=ins["out"][:, 0:1], in_=ald2[0:gmax, 0:1])
            return

        # ---------------- P7: L2 edge phase + pooling ----------------
        with tc.tile_pool(name="pool", bufs=1) as poolp, \
             tc.tile_pool(name="poolps", bufs=1, space="PSUM") as poolps:
            wpool = poolp.tile([P, NCOL * gmax], dt.float32)
            wpv = wpool[:].rearrange("p (n g) -> p n g", g=gmax)
            nc.vector.tensor_tensor(
                out=wpv,
                in0=(af("gid", NCOL).rearrange("p (n o) -> p n o", o=1)
                     .to_broadcast([P, NCOL, gmax])),
                in1=(af("iota", gmax).rearrange("p (o g) -> p o g", o=1)
                     .to_broadcast([P, NCOL, gmax])),
                op=OP.is_equal)
            nc.vector.tensor_tensor(
                out=wpv, in0=wpv,
                in1=(af("wval", NCOL).rearrange("p (n o) -> p n o", o=1)
                     .to_broadcast([P, NCOL, gmax])),
                op=OP.mult)

            pool_ps = poolps.tile([gmax, 128], dt.float32)
            _edge_phase(tc, idxs, meta, layer=2, table=table2, ald=ald2,
                        out_slot=None, wpool=wpool, pool_psum=pool_ps)

            # ---------------- P8: head ----------------
            pooled = poolp.tile([gmax, 128], dt.float32)
            nc.vector.tensor_copy(out=pooled[:], in_=pool_ps[:])
            nc.vector.tensor_tensor(out=pooled[:], in0=pooled[:],
                                    in1=af("b2g", 128, 0, gmax), op=OP.add)
            with tc.tile_pool(name="hps", bufs=1, space="PSUM") as hps:
                pT_ps = hps.tile([P, gmax], dt.float32)
                nc.tensor.transpose(out=pT_ps[:], in_=pooled[:],
                                    identity=ident[:gmax, :gmax])
                pT = poolp.tile([P, gmax], dt.float32)
                nc.vector.tensor_copy(out=pT[:], in_=pT_ps[:])
                lg_ps = hps.tile([gmax, NCLS], dt.float32)
                nc.tensor.matmul(out=lg_ps[:], lhsT=pT[:], rhs=af("fcw", NCLS),
                                 start=True, stop=True)
                lg = poolp.tile([gmax, NCLS], dt.float32)
                nc.vector.tensor_copy(out=lg[:], in_=lg_ps[:])
            nc.vector.tensor_tensor(out=lg[:], in0=lg[:],
                                    in1=af("fcb", NCLS, 0, gmax), op=OP.add)
            # log_softmax
            m = poolp.tile([gmax, 1], dt.float32)
            nc.vector.tensor_reduce(out=m[:], in_=lg[:], axis=AX.X, op=OP.max)
            nc.vector.tensor_scalar(out=lg[:], in0=lg[:], scalar1=m[:],
                                    scalar2=None, op0=OP.subtract)
            ex = poolp.tile([gmax, NCLS], dt.float32)
            nc.scalar.activation(out=ex[:], in_=lg[:], func=ACTF.Exp)
            ss = poolp.tile([gmax, 1], dt.float32)
            nc.vector.tensor_reduce(out=ss[:], in_=ex[:], axis=AX.X, op=OP.add)
            nc.scalar.activation(out=ss[:], in_=ss[:], func=ACTF.Ln)
            nc.vector.tensor_scalar(out=lg[:], in0=lg[:], scalar1=ss[:],
                                    scalar2=None, op0=OP.subtract)
            nc.sync.dma_start(out=ins["out"][:], in_=lg[:])


def _edge_phase(tc, idxs, meta, layer, table, ald, out_slot, wpool, pool_psum):
    import concourse.bass as bass
    import concourse.mybir as mybir

    nc = tc.nc
    dt = mybir.dt
    OP = mybir.AluOpType
    ACTF = mybir.ActivationFunctionType
    gmax = meta["gmax"]
    batches = meta["batches"]

    if layer == 1:
        R, F, H = 72, 64, 8
        HOFF = 64
    else:
        R, F, H = 130, 128, 1
        HOFF = 128
    C = F // H

    last = batches[-1]
    first = batches[0]

    with tc.tile_pool(name=f"ed{layer}", bufs=3 if layer == 1 else 2) as ep, \
         tc.tile_pool(name=f"eds{layer}", bufs=3) as eps:
        for (w, j0, ncols, ec0) in batches:
            Sb = ncols * w
            ed = ep.tile([P, SB * R], dt.bfloat16, tag="ed")
            for s in range(Sb):
                nc.gpsimd.indirect_dma_start(
                    out=ed[:, s * R:(s + 1) * R], out_offset=None, in_=table[:],
                    in_offset=bass.IndirectOffsetOnAxis(
                        ap=idxs[:, ec0 + s:ec0 + s + 1], axis=0))

            edk = ed[:, :Sb * R].rearrange("p (k n r) -> p k n r", k=w, r=R)
            eds_v = ed[:, :Sb * R].rearrange("p (s r) -> p s r", r=R)
            # e = al_s[src] + al_d[dst] (al_d identical across the w planes)
            et = eps.tile([P, SB * H], dt.float32, tag="et")
            etv = et[:, :Sb * H]
            nc.vector.tensor_copy(
                out=etv.rearrange("p (s h) -> p s h", h=H),
                in_=eds_v[:, :, HOFF:HOFF + H])
            aldv = (ald[:].rearrange("p (n h) -> p n h", h=H)[:, j0:j0 + ncols]
                    .rearrange("p (o n) h -> p o n h", o=1)
                    .to_broadcast([P, w, ncols, H]))
            nc.vector.tensor_tensor(
                out=etv.rearrange("p (k n h) -> p k n h", k=w, h=H),
                in0=etv.rearrange("p (k n h) -> p k n h", k=w, h=H),
                in1=aldv, op=OP.add)
            # exp(leaky_relu(e))  (leaky = max(x, 0.2x))
            lt = eps.tile([P, SB * H], dt.float32, tag="lt")
            nc.vector.tensor_scalar(out=lt[:, :Sb * H], in0=etv, scalar1=0.2,
                                    scalar2=None, op0=OP.mult)
            nc.vector.tensor_tensor(out=etv, in0=etv, in1=lt[:, :Sb * H], op=OP.max)
            nc.scalar.activation(out=etv, in_=etv, func=ACTF.Exp)
            # s[d] = sum_k exp : dense plane adds (f32)
            NH = ncols * H
            s = eps.tile([P, NB * 8], dt.float32, tag="s")
            sv = s[:, :NH]
            if w == 1:
                nc.vector.tensor_scalar(out=sv, in0=et[:, :NH], scalar1=1e-16,
                                        scalar2=None, op0=OP.add)
            else:
                nc.vector.tensor_tensor(out=sv, in0=et[:, 0:NH],
                                        in1=et[:, NH:2 * NH], op=OP.add)
                for k in range(2, w):
                    nc.vector.tensor_tensor(out=sv, in0=sv,
                                            in1=et[:, k * NH:(k + 1) * NH],
                                            op=OP.add)
                nc.vector.tensor_scalar(out=sv, in0=sv, scalar1=1e-16,
                                        scalar2=None, op0=OP.add)
            nc.vector.reciprocal(out=sv, in_=sv)
            # alpha in bf16 for the h multiply
            etb = eps.tile([P, SB * H], dt.bfloat16, tag="etb")
            nc.vector.tensor_copy(out=etb[:, :Sb * H], in_=etv)
            # WH = h[src] * alpha (in place, bf16)
            if H == 1:
                hview = eds_v[:, :, 0:F]
                exv = (etb[:, :Sb].rearrange("p (s o) -> p s o", o=1)
                       .to_broadcast([P, Sb, F]))
            else:
                hview = eds_v[:, :, 0:F].rearrange("p s (h c) -> p s h c", h=H)
                exv = (etb[:, :Sb * H].rearrange("p (s h o) -> p s h o", h=H, o=1)
                       .to_broadcast([P, Sb, H, C]))
            nc.vector.tensor_tensor(out=hview, in0=hview, in1=exv, op=OP.mult)

            # out[d] = (sum_k WH) / s[d] : pairwise bf16+bf16->f32 plane adds
            if layer == 1:
                ov = (out_slot[:].rearrange("p (n f) -> p n f", f=F)
                      [:, j0:j0 + ncols])
                x2b = None
            else:
                x2b = ep.tile([P, NB * F], dt.float32, tag="x2b")
                ov = x2b[:, :ncols * F].rearrange("p (n f) -> p n f", f=F)

            def plane(k):
                return (edk[:, k:k + 1, :, 0:F]
                        .rearrange("p o n r -> p (o n) r"))
            acc = eps.tile([P, NB * F], dt.float32, tag="acc")
            av = acc[:, :ncols * F].rearrange("p (n f) -> p n f", f=F)
            if w == 1:
                nc.vector.tensor_copy(out=ov, in_=plane(0))
            else:
                nc.vector.tensor_tensor(out=ov, in0=plane(0), in1=plane(1),
                                        op=OP.add)
                k = 2
                while k + 1 < w:
                    nc.vector.tensor_tensor(out=av, in0=plane(k),
                                            in1=plane(k + 1), op=OP.add)
                    nc.vector.tensor_tensor(out=ov, in0=ov, in1=av, op=OP.add)
                    k += 2
                if k < w:
                    nc.vector.tensor_copy(out=av, in_=plane(k))
                    nc.vector.tensor_tensor(out=ov, in0=ov, in1=av, op=OP.add)
            if H == 1:
                sinvv = (sv.rearrange("p (n o) -> p n o", o=1)
                         .to_broadcast([P, ncols, F]))
                ovv = ov
            else:
                sinvv = (sv.rearrange("p (n h o) -> p n h o", h=H, o=1)
                         .to_broadcast([P, ncols, H, C]))
                ovv = ov.rearrange("p n (h c) -> p n h c", h=H)
            nc.vector.tensor_tensor(out=ovv, in0=ovv, in1=sinvv, op=OP.mult)

            if layer == 2:
                for jj in range(ncols):
                    nc.tensor.matmul(
                        out=pool_psum[:],
                        lhsT=wpool[:, (j0 + jj) * gmax:(j0 + jj + 1) * gmax],
                        rhs=x2b[:, jj * F:(jj + 1) * F],
                        start=((w, j0, ncols, ec0) == first and jj == 0),
                        stop=((w, j0, ncols, ec0) == last and jj == ncols - 1),
                        skip_group_check=True)


# ----------------------------------------------------------------------------
# runner
# ----------------------------------------------------------------------------

_CACHE = {}


def _get_nc(meta, in_map0):
    key = str(sorted(meta.items(), key=lambda kv: kv[0]))
    if key in _CACHE:
        return _CACHE[key]
    import concourse.bacc as bacc
    import concourse.tile as tile
    import concourse.mybir as mybir
    dt = mybir.dt
    nc = bacc.Bacc("TRN2", target_bir_lowering=False, debug=False,
                   num_devices=NCORES)
    ins = {}
    for name, arr in in_map0.items():
        ins[name] = nc.dram_tensor(name, list(arr.shape),
                                   _np_dtype_to_bir(arr),
                                   kind="ExternalInput").ap()
    ins["out"] = nc.dram_tensor("out", [meta["gmax"], NCLS], dt.float32,
                                kind="ExternalOutput").ap()
    with tile.TileContext(nc) as tc:
        build_program(tc, ins, meta)
    nc.compile()
    _CACHE[key] = nc
    return nc


def _np_dtype_to_bir(a):
    import concourse.mybir as mybir
    import ml_dtypes
    dt = mybir.dt
    if a.dtype == np.int32:
        return dt.int32
    if a.dtype == ml_dtypes.bfloat16:
        return dt.bfloat16
    return dt.float32


def make_inputs(x, edge_index, batch, W1, a_src1, a_dst1, b1, W2, a_src2,
                a_dst2, b2, fc_w, fc_b):
    import ml_dtypes
    x = np.asarray(x, np.float32)
    host, meta, auxd = _prep(x, np.asarray(edge_index), np.asarray(batch))
    gmax = meta["gmax"]
    ncu = meta["ncu"]
    lay = _aux_layout(meta)
    n0 = meta["n0"]

    ge = np.searchsorted(np.asarray(batch), np.arange(NG), side="left")
    gEnd = np.searchsorted(np.asarray(batch), np.arange(NG), side="right")

    in_maps = []
    for c in range(NCORES):
        h = host[c]
        aux = np.zeros((P, lay["_total"]), np.float32)

        def put(name, arr, p0=0):
            arr = np.asarray(arr, np.float32)
            aux[p0:p0 + arr.shape[0], lay[name]:lay[name] + arr.shape[1]] = arr

        put("gid", h["gid"])
        put("wval", h["wval"])
        put("iota", np.tile(np.arange(gmax, dtype=np.float32).reshape(1, gmax),
                            (P, 1)))
        aux[:, lay["own"]:lay["own"] + 1] = h["ownbase"].view(np.float32)
        g0c, g1c = meta["g0"][c], meta["g1"][c]
        nonempty = np.zeros((gmax, 1), np.float32)
        cntc = (gEnd - ge)[g0c:g1c]
        nonempty[:g1c - g0c, 0] = (cntc > 0).astype(np.float32)
        put("b2g", nonempty * np.asarray(b2, np.float32).reshape(1, 128))
        put("fcb", np.tile(np.asarray(fc_b, np.float32).reshape(1, NCLS),
                           (gmax, 1)))
        put("W1", np.asarray(W1, np.float32))
        put("a1s", np.tile(np.asarray(a_src1, np.float32).reshape(1, 64), (P, 1)))
        put("a1d", np.tile(np.asarray(a_dst1, np.float32).reshape(1, 64), (P, 1)))
        put("b1", np.tile(np.asarray(b1, np.float32).reshape(1, 64), (P, 1)))
        put("W2", np.asarray(W2, np.float32))
        put("a2s", np.tile(np.asarray(a_src2, np.float32).reshape(1, 128), (64, 1)))
        put("a2d", np.tile(np.asarray(a_dst2, np.float32).reshape(1, 128), (64, 1)))
        put("fcw", np.asarray(fc_w, np.float32))
        aux[:, lay["idx"]:lay["idx"] + meta["S"]] = h["idx"].view(np.float32)

        # x permuted into slot order, bf16, [feat, slot], used columns only
        sn = auxd["slot_nodes"][c]
        valid = sn >= 0
        xs = np.zeros((ncu * P, x.shape[1]), np.float32)
        ppi, jji = np.nonzero(valid)
        xs[jji * P + ppi] = x[n0[c] + sn[ppi, jji]]
        xT = np.ascontiguousarray(xs.T).astype(ml_dtypes.bfloat16)

        in_maps.append(dict(xT=xT, aux=aux))
    return in_maps, meta, auxd


def kernel(x, edge_index, batch, W1, a_src1, a_dst1, b1, W2, a_src2, a_dst2,
           b2, fc_w, fc_b):
    in_maps, meta, auxd = make_inputs(x, edge_index, batch, W1, a_src1, a_dst1,
                                      b1, W2, a_src2, a_dst2, b2, fc_w, fc_b)
    global _LAST
    _LAST = dict(meta=meta, aux=auxd)
    nc = _get_nc(meta, in_maps[0])
    from concourse.bass_utils import run_bass_kernel_spmd
    res = run_bass_kernel_spmd(nc, in_maps, core_ids=list(range(NCORES)))
    _LAST["res"] = res
    out = np.zeros((NG, NCLS), np.float32)
    for c in range(NCORES):
        g0, g1 = meta["g0"][c], meta["g1"][c]
        out[g0:g1] = res.results[c]["out"][:g1 - g0]
    return out
